# revision 1
# baseline (speedup 1.0000x reference)
"""DANetHead Trainium2 kernel: 8-core SPMD (batch x row-half sharding).

Self-contained: hardcodes all shapes from the problem spec.

Per-core layout (core c: sample b=c//2, half h=c%2):
  P = [-1, 0..63, 64] (66 padded rows; -1/64 zero).
  x_pad rows R=0..67 hold padded row P[(R-1+32h) % 66]  (cyclic rotation, so
  every core's attention/conv2 window is local rows 0..33 uniformly).
  conv1 output local row L (0..65) centers on P[(L+32h) % 66].
  window = local rows 0..33 (flat 0..2175); my output rows = 1..32.
"""
import numpy as np

import concourse.bass as bass
import concourse.tile as tile
from concourse import bacc, mybir
from concourse.bass_utils import run_bass_kernel_spmd

F32 = mybir.dt.float32
F32R = mybir.dt.float32r
BF16 = mybir.dt.bfloat16
AF = mybir.ActivationFunctionType
ALU = mybir.AluOpType

B, CIN, H, W = 4, 256, 64, 64
CI, CQ, CO = 64, 8, 256
NCORES = 8
LR = 66                  # local feat1 rows
NP = LR * W              # 4224
NJT = NP // 128          # 33 j-tiles
WIN = 34 * W             # 2176
MY = 32 * W              # 2048
XR, XC = 68, 66          # x_pad rows/cols
NTAPS = 18               # 9 taps x 2 cin blocks
# i chunks: CAM uses full window; PAM main loop uses ICM + bf16 tail
IC = [(0, 512), (512, 512), (1024, 512), (1536, 512), (2048, 128)]
ICM = [(0, 512), (512, 512), (1024, 512), (1536, 384), (1920, 256)]
# conv1 output tiles: (row0, nrows, chunk)
C1T = [(8 * T, 8, T) for T in range(8)] + [(64, 2, 8)]
C1GRP = [(0, 1), (2, 3), (4, 5), (6, 7, 8)]
XCHUNK = [(8 * T, 10) for T in range(8)] + [(64, 4)]  # (row0, nrows)
N_STAT = 16384.0


# ---------------------------------------------------------------- host prep
def _rot_centers(h):
    P = [-1] + list(range(64)) + [64]
    return [P[(L + 32 * h) % 66] for L in range(LR)]


def _prep_core_inputs(x, w1, bn_g, bn_b, wq, bq, wk, bk, wv, bv,
                      gamma_pam, gamma_cam, w2, w8, b8):
    f = np.float32
    # shared weights
    w1s = np.zeros((128, NTAPS, CI), f)
    for dy in range(3):
        for dx in range(3):
            for cb in range(2):
                s = (dy * 3 + dx) * 2 + cb
                w1s[:, s, :] = w1[:, cb * 128:(cb + 1) * 128, dy, dx].T
    wqkv = np.zeros((65, 80), f)
    wqkv[:64, 0:64] = wv[:, :, 0, 0].T
    wqkv[:64, 64:72] = wq[:, :, 0, 0].T
    wqkv[:64, 72:80] = wk[:, :, 0, 0].T
    wqkv[64, 0:64] = bv
    wqkv[64, 64:72] = bq
    wqkv[64, 72:80] = bk
    w2a = np.zeros((128, 3, CI), f)
    w2b = np.zeros((64, 3, CI), f)
    for dx in range(3):
        w2a[:64, dx, :] = w2[:, :, 0, dx].T
        w2a[64:, dx, :] = w2[:, :, 1, dx].T
        w2b[:, dx, :] = w2[:, :, 2, dx].T
    w8s = np.zeros((65, 2, 128), f)
    for blk in range(2):
        w8s[:64, blk, :] = w8[blk * 128:(blk + 1) * 128, :, 0, 0].T
        w8s[64, blk, :] = b8[blk * 128:(blk + 1) * 128]
    bngb = np.stack([bn_g, bn_b], 1).astype(f)
    consts = np.array([[float(gamma_pam[0]), float(gamma_cam[0])]], f)
    iden = np.eye(128, dtype=f)

    shared = dict(w1s=w1s, wqkv=wqkv, w2a=w2a.reshape(128, 3 * CI),
                  w2b=w2b.reshape(64, 3 * CI), w8s=w8s.reshape(65, 256),
                  bngb=bngb, consts=consts, iden=iden)

    in_maps = []
    for c in range(NCORES):
        b, h = divmod(c, 2)
        # x_pad [128, 2, 68, 66]
        P = [-1] + list(range(64)) + [64]
        rows = [P[(R - 1 + 32 * h) % 66] for R in range(XR)]
        xp = np.zeros((128, 2, XR, XC), f)
        for R, g in enumerate(rows):
            if 0 <= g <= 63:
                xr = x[b, :, g, :]                       # [256, 64]
                xp[:, 0, R, 1:65] = xr[:128]
                xp[:, 1, R, 1:65] = xr[128:]
        centers = _rot_centers(h)
        real = np.array([0 <= g <= 63 for g in centers])
        realp = np.repeat(real, W)                        # [4224]
        ebias = np.stack([np.where(realp, 0.0, -1000.0).astype(f),
                          np.ones(NP, f)])
        nmask = np.where(realp, 1.0, 0.0).astype(f).reshape(NJT, 128).T.copy()
        hmask = np.zeros((64, 2), f)
        hmask[:, 0] = 0.0 if h == 0 else 1.0
        hmask[:, 1] = 0.0 if h == 1 else 1.0
        m = dict(shared)
        m.update(xp=xp, ebias=ebias, nmask=nmask, hmask=hmask)
        in_maps.append(m)
    return in_maps


# ---------------------------------------------------------------- bass build
def _build(nreps=1):
    nc = bacc.Bacc()
    xp = nc.declare_dram_parameter("xp", [128, 2, XR, XC], F32R, isOutput=False)
    w1s = nc.declare_dram_parameter("w1s", [128, NTAPS, CI], F32R, isOutput=False)
    wqkv = nc.declare_dram_parameter("wqkv", [65, 80], F32R, isOutput=False)
    w2a = nc.declare_dram_parameter("w2a", [128, 3 * CI], F32R, isOutput=False)
    w2b = nc.declare_dram_parameter("w2b", [64, 3 * CI], F32R, isOutput=False)
    w8s = nc.declare_dram_parameter("w8s", [65, 256], F32R, isOutput=False)
    bngb = nc.declare_dram_parameter("bngb", [64, 2], F32, isOutput=False)
    ebias = nc.declare_dram_parameter("ebias", [2, NP], F32R, isOutput=False)
    nmask = nc.declare_dram_parameter("nmask", [128, NJT], F32, isOutput=False)
    hmask = nc.declare_dram_parameter("hmask", [64, 2], F32, isOutput=False)
    consts = nc.declare_dram_parameter("consts", [1, 2], F32, isOutput=False)
    iden = nc.declare_dram_parameter("iden", [128, 128], F32R, isOutput=False)
    out = nc.declare_dram_parameter("out", [256, MY], F32, isOutput=True)

    with tile.TileContext(nc) as tc:
        with tc.tile_pool(name="big", bufs=1) as big, \
             tc.tile_pool(name="xin", bufs=2) as xin, \
             tc.tile_pool(name="wt", bufs=1) as wt, \
             tc.tile_pool(name="sm", bufs=2) as sm, \
             tc.tile_pool(name="et", bufs=2) as etp, \
             tc.tile_pool(name="ps", bufs=2, space="PSUM") as ps, \
             tc.tile_pool(name="pt", bufs=2, space="PSUM") as ptp, \
             tc.tile_pool(name="mc", bufs=2, space="PSUM") as mcp, \
             tc.tile_pool(name="dram", bufs=1, space="DRAM") as dram:

            # ---- persistent sbuf tensors
            feat = big.tile([65, NP], F32R, tag="feat")   # y1 then feat1(+ones)
            qkv = big.tile([80, NP], F32R, tag="qkv")
            qr = big.tile([128, WIN], F32R, tag="qr")
            kr4 = big.tile([128, 9, 128], F32R, tag="kr4")
            vT = big.tile([128, NJT, 65], F32R, tag="vT")
            fT = big.tile([128, NJT, CI], F32R, tag="fT")
            sabuf = big.tile([128, 34, XC], F32R, tag="sabuf")
            scbuf = big.tile([128, 34, XC], F32R, tag="scbuf")
            y2a = big.tile([64, MY], F32, tag="y2a")
            y2b = big.tile([64, MY], F32, tag="y2b")
            fsum = big.tile([65, MY], F32R, tag="fsum")
            pacc = big.tile([65, WIN], F32, tag="pacc")   # pam accumulator

            # ---- weights / consts in sbuf
            w1t = wt.tile([128, NTAPS, CI], F32R, tag="w1t")
            wqkvt = wt.tile([65, 80], F32R, tag="wqkvt")
            w2at = wt.tile([128, 3 * CI], F32R, tag="w2at")
            w2bt = wt.tile([64, 3 * CI], F32R, tag="w2bt")
            w8t = wt.tile([65, 256], F32R, tag="w8t")
            bngbt = wt.tile([64, 2], F32, tag="bngbt")
            nmt = wt.tile([128, NJT], F32, tag="nmt")
            hmt = wt.tile([64, 2], F32, tag="hmt")
            cst = wt.tile([1, 2], F32, tag="cst")
            gcam = wt.tile([64, 1], F32, tag="gcam")
            epst = wt.tile([64, 1], F32, tag="epst")
            nc.vector.memset(epst, 1e-5)
            idt = wt.tile([128, 128], F32R, tag="idt")
            nc.sync.dma_start(out=w1t, in_=w1s[:, :, :])
            nc.sync.dma_start(out=wqkvt, in_=wqkv[:, :])
            nc.sync.dma_start(out=w2at, in_=w2a[:, :])
            nc.sync.dma_start(out=w2bt, in_=w2b[:, :])
            nc.sync.dma_start(out=w8t, in_=w8s[:, :])
            nc.sync.dma_start(out=bngbt, in_=bngb[:, :])
            nc.sync.dma_start(out=nmt, in_=nmask[:, :])
            nc.sync.dma_start(out=hmt, in_=hmask[:, :])
            nc.sync.dma_start(out=cst, in_=consts[:, :])
            nc.sync.dma_start(out=idt, in_=iden[:, :])
            gc_src = bass.AP(tensor=consts, offset=1, ap=[[0, 64], [1, 1]])
            nc.gpsimd.dma_start(out=gcam, in_=gc_src)
            nc.gpsimd.memset(feat[64:65, :].bitcast(F32), 1.0)
            nc.gpsimd.memset(fsum[64:65, :].bitcast(F32), 1.0)
            nc.gpsimd.memset(kr4[:, :, :].bitcast(F32), 0.0)
            nc.gpsimd.memset(vT[:, :, 64:65].bitcast(F32), 1.0)
            for bf in (sabuf, scbuf):
                nc.gpsimd.memset(bf[0:64, :, 0:1].bitcast(F32), 0.0)
                nc.gpsimd.memset(bf[0:64, :, 65:66].bitcast(F32), 0.0)

            def _body(rep):
                # ---- x chunks
                xc = []
                for (r0, nr) in XCHUNK:
                    t = xin.tile([128, 2, nr, XC], F32R, tag=f"xc{nr}",
                                 name=f"xc{r0}", bufs=3 if nr == 10 else 1)
                    nc.sync.dma_start(out=t, in_=xp[:, :, r0:r0 + nr, :])
                    xc.append(t)

                # ---- conv1 -> feat rows 0..63 hold raw y1
                stats1 = sm.tile([64, 5, 6], F32, tag="stats1")
                stat_slices = [(0, 64, 448), (1, 0, 512), (2, 0, 512),
                               (3, 0, 512), (4, 0, 64)]
                for grp in C1GRP:
                    pst = {}
                    for T in grp:
                        r0, nr, ci_ = C1T[T]
                        pst[T] = mcp.tile([64, nr * W], F32, tag="mc",
                                          name=f"c1ps{T}")
                    for s in range(NTAPS):
                        tap, cb = divmod(s, 2)
                        dy, dx = divmod(tap, 3)
                        for T in grp:
                            r0, nr, ci_ = C1T[T]
                            rhs = xc[ci_][:, cb, dy:dy + nr, dx:dx + 64]
                            nc.tensor.matmul(pst[T], w1t[:, s, :], rhs,
                                             start=(s == 0), stop=(s == NTAPS - 1))
                    for T in grp:
                        r0, nr, ci_ = C1T[T]
                        nc.vector.tensor_copy(feat[0:64, r0 * W:(r0 + nr) * W],
                                              pst[T])
                for (k, off, ln) in stat_slices:
                    T0 = [0, 512, 1024, 1536, 2048][k]
                    nc.vector.bn_stats(stats1[:, k, :],
                                       feat[0:64, T0 + off:T0 + off + ln])
                mv1 = sm.tile([64, 2], F32, tag="mv1")
                nc.vector.bn_aggr(mv1, stats1[:, :, :])

                def bn_coeffs(gl, tag):
                    """gl [64,2] = (sum, sumsq) -> (scale, shift) [64,1] f32."""
                    mean = sm.tile([64, 1], F32, tag=tag + "m", name=tag + "m")
                    var = sm.tile([64, 1], F32, tag=tag + "v", name=tag + "v")
                    scl = sm.tile([64, 1], F32, tag=tag + "s", name=tag + "s")
                    sh = sm.tile([64, 1], F32, tag=tag + "h", name=tag + "h")
                    nc.vector.tensor_scalar_mul(mean, gl[:, 0:1], 1.0 / N_STAT)
                    nc.vector.tensor_scalar_mul(var, gl[:, 1:2], 1.0 / N_STAT)
                    nc.vector.tensor_tensor(scl, mean, mean, ALU.mult)
                    nc.vector.tensor_tensor(var, var, scl, ALU.subtract)
                    nc.scalar.activation(var, var, AF.Sqrt, bias=epst, scale=1.0)
                    nc.vector.reciprocal(var, var)
                    nc.vector.tensor_tensor(scl, bngbt[:, 0:1], var, ALU.mult)
                    nc.vector.tensor_tensor(sh, mean, scl, ALU.mult)
                    nc.vector.tensor_tensor(sh, bngbt[:, 1:2], sh, ALU.subtract)
                    return scl, sh

                def stat_ar(mv, tag):
                    """partial (mean,var over MY) -> AllReduce -> (sum,sumsq)."""
                    ars = sm.tile([64, 2], F32, tag=tag + "s", name=tag + "s")
                    t_t = sm.tile([64, 1], F32, tag=tag + "t", name=tag + "t")
                    nc.vector.tensor_scalar_mul(ars[:, 0:1], mv[:, 0:1], float(MY))
                    nc.vector.tensor_tensor(t_t, mv[:, 0:1], mv[:, 0:1], ALU.mult)
                    nc.vector.tensor_tensor(t_t, mv[:, 1:2], t_t, ALU.add)
                    nc.vector.tensor_scalar_mul(ars[:, 1:2], t_t, float(MY))
                    a_in = dram.tile([64, 2], F32, tag=tag + "_in",
                                     name=tag + "_in")
                    a_out = dram.tile([64, 2], F32, tag=tag + "_out",
                                      name=tag + "_out")
                    nc.sync.dma_start(out=a_in[:, :], in_=ars)
                    nc.gpsimd.collective_compute(
                        "AllReduce", ALU.add,
                        replica_groups=[list(range(NCORES))],
                        ins=[a_in.opt()], outs=[a_out.opt()])
                    gl = sm.tile([64, 2], F32, tag=tag + "g", name=tag + "g")
                    nc.sync.dma_start(out=gl, in_=a_out[:, :])
                    return gl

                # AR1: bn1 stats
                gl1 = stat_ar(mv1, "ar1")
                sc1, sh1 = bn_coeffs(gl1, "bn1")
                for (r0, nr, _) in C1T:
                    sl = feat[0:64, r0 * W:(r0 + nr) * W]
                    nc.scalar.activation(sl, sl, AF.Relu, bias=sh1, scale=sc1)

                # ---- qkv
                qkvtiles = [(t * 512, 512) for t in range(8)] + [(4096, 128)]
                for ti, (c0, cw) in enumerate(qkvtiles):
                    qps = mcp.tile([80, cw], F32, tag="mc", name="qps")
                    nc.tensor.matmul(qps, wqkvt, feat[:, c0:c0 + cw],
                                     start=True, stop=True)
                    nc.vector.tensor_copy(qkv[:, c0:c0 + cw], qps)
                # qr: q replicated at partition groups; row 32g+8 = ones
                # (pairs with the ebias row in kr4 -> energy gets +ebias[j])
                for g in range(4):
                    nc.sync.dma_start(out=qr[32 * g:32 * g + 8, :],
                                      in_=qkv[64:72, 0:WIN])
                for g in range(4):
                    nc.sync.dma_start(out=qr[32 * g + 8:32 * g + 9, :],
                                      in_=ebias[1:2, 0:WIN])
                # kr4: k repartitioned per j-group; row 8 of each 32-block holds
                # the exp masking bias for that j-tile
                kr4r = kr4.rearrange("(g p) t n -> g p t n", p=32)
                kbounce = dram.tile([8, NP], F32R, tag="kbounce", name="kbounce")
                nc.sync.dma_start(out=kbounce[:, :], in_=qkv[72:80, :])
                for u in range(4):
                    ksrc = bass.AP(tensor=kbounce.tensor,
                                   offset=kbounce.offset + u * 128,
                                   ap=[[NP, 8], [512, 8], [1, 128]])
                    nc.sync.dma_start(out=kr4[32 * u:32 * u + 8, 0:8, :],
                                      in_=ksrc)
                    bsrc = bass.AP(tensor=ebias, offset=u * 128,
                                   ap=[[512, 8], [1, 128]])
                    nc.sync.dma_start(out=kr4[32 * u + 8:32 * u + 9, 0:8, :],
                                      in_=bsrc)
                nc.sync.dma_start(out=kr4[0:8, 8, :], in_=kbounce[:, 4096:4224])
                nc.sync.dma_start(out=kr4[8:9, 8, :], in_=ebias[0:1, 4096:4224])

                # ---- vT transpose (+ones col), 4 per psum bank
                for j0 in range(0, 32, 4):
                    tp = mcp.tile([128, 4, 64], F32R, tag="mc",
                                  name=f"vtp{j0}")
                    for k in range(4):
                        jt = j0 + k
                        nc.tensor.transpose(
                            tp[:, k, :],
                            qkv[0:64, jt * 128:(jt + 1) * 128],
                            idt[0:64, 0:64])
                    nc.vector.tensor_copy(vT[:, j0:j0 + 4, 0:64], tp)
                tpl = mcp.tile([128, 64], F32R, tag="mc", name="vtpl")
                nc.tensor.transpose(tpl, qkv[0:64, 32 * 128:33 * 128],
                                    idt[0:64, 0:64])
                nc.vector.tensor_copy(vT[:, 32, 0:64], tpl)

                # ================= interleaved attention + CAM emission ========
                def pam_pair(jg0, chunk_cb=None):
                    """Emit energy/exp/pam for j-groups jg0, jg0+1 (or lone 8)."""
                    jgs = [jg0] if jg0 == 8 else [jg0, jg0 + 1]
                    for ici, (i0, iw) in enumerate(ICM):
                        pt = ptp.tile([65, iw], F32, tag="pt", name="pt")
                        nmm = sum(4 if j < 8 else 1 for j in jgs)
                        k = 0
                        for jg in jgs:
                            nu2 = 2 if jg < 8 else 1
                            for p in range(2 if jg < 8 else 1):
                                et_ps = ps.tile([128, 2, 512], F32, tag="ps",
                                                name="et_ps")
                                for u2 in range(nu2):
                                    u = 2 * p + u2
                                    nc.tensor.matmul(
                                        et_ps[:, u2, 0:iw],
                                        kr4[32 * u:32 * u + 32, jg, :],
                                        qr[32 * u:32 * u + 32, i0:i0 + iw],
                                        start=True, stop=True,
                                        tile_position=(32 * u, 0))
                                eT = etp.tile([128, 2, 512], F32R, tag="et",
                                              bufs=2, name="eT")
                                if nu2 == 2:
                                    nc.scalar.activation(eT[:, :, 0:iw],
                                                         et_ps[:, :, 0:iw],
                                                         AF.Exp, bias=0.0,
                                                         scale=1.0)
                                else:
                                    nc.scalar.activation(eT[:, 0, 0:iw],
                                                         et_ps[:, 0, 0:iw],
                                                         AF.Exp, bias=0.0,
                                                         scale=1.0)
                                for u2 in range(nu2):
                                    jt = 4 * jg + 2 * p + u2
                                    nc.tensor.matmul(pt, vT[:, jt, :],
                                                     eT[:, u2, 0:iw],
                                                     start=(k == 0),
                                                     stop=(k == nmm - 1))
                                    k += 1
                        if jg0 == 0:
                            nc.vector.tensor_copy(pacc[:, i0:i0 + iw], pt)
                        else:
                            nc.vector.tensor_tensor(pacc[:, i0:i0 + iw],
                                                    pacc[:, i0:i0 + iw], pt,
                                                    ALU.add)
                        if chunk_cb is not None:
                            chunk_cb(ici, i0, iw)

                pam_pair(0)
                # fT transposes (CAM input), masked
                for jt in range(NJT):
                    tp = mcp.tile([128, 64], F32R, tag="mc", name=f"ftp{jt}")
                    nc.tensor.transpose(tp, feat[0:64, jt * 128:(jt + 1) * 128],
                                        idt[0:64, 0:64])
                    nc.vector.tensor_scalar_mul(fT[:, jt, :], tp, nmt[:, jt:jt + 1])

                pam_pair(2)
                # CAM: ce (chunked), softmax, cattnT
                ce_sb = sm.tile([64, 64], F32, tag="ce_sb")
                for ci_, (j0, nj) in enumerate([(0, 9), (9, 8), (17, 8), (25, 8)]):
                    ce_ps = mcp.tile([64, 64], F32, tag="mc", name=f"ce{ci_}")
                    for k in range(nj):
                        jt = j0 + k
                        nc.tensor.matmul(ce_ps, fT[:, jt, :], fT[:, jt, :],
                                         start=(k == 0), stop=(k == nj - 1))
                    if ci_ == 0:
                        nc.vector.tensor_copy(ce_sb, ce_ps)
                    else:
                        nc.vector.tensor_tensor(ce_sb, ce_sb, ce_ps, ALU.add)
                rmin = sm.tile([64, 1], F32, tag="rmin")
                nc.vector.tensor_reduce(rmin, ce_sb, mybir.AxisListType.X, ALU.min)
                cu = sm.tile([64, 64], F32, tag="cu")
                nc.scalar.activation(cu, ce_sb, AF.Exp, bias=rmin, scale=-1.0)
                rs = sm.tile([64, 1], F32, tag="rs")
                nc.vector.tensor_reduce(rs, cu, mybir.AxisListType.X, ALU.add)
                nc.vector.reciprocal(rs, rs)
                cattn = sm.tile([64, 64], F32R, tag="cattn")
                nc.vector.tensor_scalar_mul(cattn, cu, rs)
                ctp = mcp.tile([64, 64], F32R, tag="mc", name="ctp")
                nc.tensor.transpose(ctp, cattn, idt[0:64, 0:64])
                cattnT = sm.tile([64, 64], F32R, tag="cattnT")
                nc.vector.tensor_copy(cattnT, ctp)

                pam_pair(4)
                # CAM apply + scbuf
                for (i0, iw) in IC:
                    cam_ps = mcp.tile([64, iw], F32, tag="mc", name="cam_ps")
                    nc.tensor.matmul(cam_ps, cattnT, feat[0:64, i0:i0 + iw],
                                     start=True, stop=True)
                    tmpc = etp.tile([64, iw], F32R, tag="camt", bufs=3,
                                    name="tmpc")
                    nc.vector.tensor_scalar_mul(tmpc, cam_ps, gcam)
                    r0, nr = i0 // W, iw // W
                    nc.vector.tensor_tensor(
                        scbuf[0:64, r0:r0 + nr, 1:65],
                        tmpc[:, :].rearrange("p (r c) -> p r c", c=W),
                        feat[0:64, i0:i0 + iw].rearrange("p (r c) -> p r c", c=W),
                        ALU.add)
                nc.vector.tensor_scalar_mul(scbuf[0:64, 0, 1:65],
                                            scbuf[0:64, 0, 1:65], hmt[:, 0:1])
                nc.vector.tensor_scalar_mul(scbuf[0:64, 33, 1:65],
                                            scbuf[0:64, 33, 1:65], hmt[:, 1:2])
                for (a, b) in [(0, 9), (9, 17), (17, 25), (25, 33)]:
                    nc.gpsimd.tensor_copy(scbuf[64:128, a:b, :],
                                          scbuf[0:64, a + 1:b + 1, :])

                def conv2(buf, y2sb, sttag):
                    st = sm.tile([64, 4, 6], F32, tag=sttag, name=sttag)
                    for T in range(4):
                        r0 = 1 + 8 * T
                        yps = mcp.tile([64, 512], F32, tag="mc", name="yps")
                        for dxi in range(3):
                            rhs1 = buf[:, r0 - 1:r0 + 7, dxi:dxi + 64]
                            nc.tensor.matmul(yps, w2at[:, dxi * 64:(dxi + 1) * 64],
                                             rhs1, start=(dxi == 0), stop=False)
                            rhs2 = buf[0:64, r0 + 1:r0 + 9, dxi:dxi + 64]
                            nc.tensor.matmul(yps, w2bt[:, dxi * 64:(dxi + 1) * 64],
                                             rhs2, start=False, stop=(dxi == 2))
                        nc.vector.bn_stats(st[:, T, :], yps)
                        nc.vector.tensor_copy(y2sb[:, T * 512:(T + 1) * 512], yps)
                    mv = sm.tile([64, 2], F32, tag=sttag + "mv", name=sttag + "mv")
                    nc.vector.bn_aggr(mv, st[:, :, :])
                    return mv

                pam_pair(6)
                # conv2 on CAM branch + its stats AR (hidden under attention)
                mvb = conv2(scbuf, y2b, "stb")
                glb = stat_ar(mvb, "arb")
                scb, shb = bn_coeffs(glb, "bnb")
                rb = big.tile([64, MY], F32R, tag="rb")
                nc.scalar.activation(rb, y2b, AF.Relu, bias=shb, scale=scb)

                # ---- pam normalize (r = gamma_pam / s), sa = pam_u*r + feat1
                def pam_div(src, i0, iw, sfx):
                    r32 = sm.tile([1, iw], F32, tag="r32", name="r32" + sfx)
                    nc.vector.reciprocal(r32, src[64:65, :])
                    rr = sm.tile([1, iw], F32R, tag="rr", name="rr" + sfx)
                    nc.vector.tensor_scalar_mul(rr, r32, cst[0:1, 0:1])
                    rbc = etp.tile([64, iw], F32R, tag="camt", bufs=3,
                                   name="rbc" + sfx)
                    nc.gpsimd.partition_broadcast(rbc, rr)
                    tmpa = etp.tile([64, iw], F32R, tag="camt", bufs=3,
                                    name="tmpa" + sfx)
                    nc.vector.tensor_tensor(tmpa, src[0:64, :], rbc, ALU.mult)
                    r0, nr = i0 // W, iw // W
                    nc.vector.tensor_tensor(
                        sabuf[0:64, r0:r0 + nr, 1:65],
                        tmpa[:, :].rearrange("p (r c) -> p r c", c=W),
                        feat[0:64, i0:i0 + iw].rearrange("p (r c) -> p r c", c=W),
                        ALU.add)

                pam_pair(8, chunk_cb=lambda ici, i0, iw: pam_div(
                    pacc[:, i0:i0 + iw], i0, iw, str(ici)))
                nc.vector.tensor_scalar_mul(sabuf[0:64, 0, 1:65],
                                            sabuf[0:64, 0, 1:65], hmt[:, 0:1])
                nc.vector.tensor_scalar_mul(sabuf[0:64, 33, 1:65],
                                            sabuf[0:64, 33, 1:65], hmt[:, 1:2])
                for (a, b) in [(0, 9), (9, 17), (17, 25), (25, 33)]:
                    nc.gpsimd.tensor_copy(sabuf[64:128, a:b, :],
                                          sabuf[0:64, a + 1:b + 1, :])

                mva = conv2(sabuf, y2a, "sta")
                gla = stat_ar(mva, "ara")
                sca, sha = bn_coeffs(gla, "bna")

                # ---- relu + sum + conv8, pipelined per 512 chunk
                for T in range(4):
                    sl = slice(T * 512, (T + 1) * 512)
                    ra = etp.tile([64, 512], F32R, tag="camt", bufs=3,
                                  name=f"ra{T}")
                    nc.scalar.activation(ra, y2a[:, sl], AF.Relu,
                                         bias=sha, scale=sca)
                    nc.vector.tensor_tensor(fsum[0:64, sl], ra, rb[:, sl], ALU.add)
                    for blk in range(2):
                        ops_ = mcp.tile([128, 512], F32, tag="mc", name="ops")
                        nc.tensor.matmul(ops_, w8t[:, blk * 128:(blk + 1) * 128],
                                         fsum[:, sl], start=True, stop=True)
                        osb = etp.tile([128, 512], F32, tag="camt", bufs=3,
                                       name="osb")
                        nc.vector.tensor_copy(osb, ops_)
                        nc.sync.dma_start(out=out[blk * 128:(blk + 1) * 128, sl],
                                          in_=osb)

            for rep in range(nreps):
                _body(rep)
    nc.finalize()
    return nc


_NC_CACHE = {}


def kernel(**inputs):
    if "nc" not in _NC_CACHE:
        _NC_CACHE["nc"] = _build()
    nc = _NC_CACHE["nc"]
    x = np.asarray(inputs["x"], np.float32)
    in_maps = _prep_core_inputs(
        x, np.asarray(inputs["w1"]), np.asarray(inputs["bn_g"]),
        np.asarray(inputs["bn_b"]), np.asarray(inputs["wq"]),
        np.asarray(inputs["bq"]), np.asarray(inputs["wk"]),
        np.asarray(inputs["bk"]), np.asarray(inputs["wv"]),
        np.asarray(inputs["bv"]), np.asarray(inputs["gamma_pam"]),
        np.asarray(inputs["gamma_cam"]), np.asarray(inputs["w2"]),
        np.asarray(inputs["w8"]), np.asarray(inputs["b8"]))
    res = run_bass_kernel_spmd(nc, in_maps, list(range(NCORES)))
    out = np.zeros((B, CO, H, W), np.float32)
    for c in range(NCORES):
        b, h = divmod(c, 2)
        out[b, :, 32 * h:32 * h + 32, :] = \
            res.results[c]["out"].reshape(CO, 32, W)
    return out



# revision 4
# speedup vs baseline: 9.7079x; 9.7079x over previous
"""DANetHead Trainium2 kernel: 8-core SPMD (batch x row-half sharding).

Self-contained: hardcodes all shapes from the problem spec.

I/O-optimized vs the v1 baseline (wall time through the axon tunnel is
dominated by host<->device bytes, not device compute):
  * x is exact-sharded: each core uploads only its 32 rows of its sample
    in fp16 ([128, 2, 32, 64], 1.05 MB); the full sample is rebuilt on
    device with a pair AllGather.
  * The h-dependent row rotation (the per-core attention window trick)
    cannot pass through the shared AllGather (both pair cores run the
    same code on the same gathered bytes), so BOTH rotation patterns are
    assembled in SBUF and blended with a per-core 0/1 mask (xsel).
  * conv1 runs in fp16 (weights uploaded fp16).
  * The final 1x1 conv8 (64->256 ch) is moved to the host: the device
    returns the 64-channel feat_sum in fp16 (0.26 MB/core instead of
    2 MB/core), and numpy applies W8 @ feat_sum + b8.

Per-core layout (core c: sample b=c//2, half h=c%2):
  P = [-1, 0..63, 64] (66 padded rows; -1/64 zero).
  x_pad rows R=0..67 hold padded row P[(R-1+32h) % 66]  (cyclic rotation, so
  every core's attention/conv2 window is local rows 0..33 uniformly).
  conv1 output local row L (0..65) centers on P[(L+32h) % 66].
  window = local rows 0..33 (flat 0..2175); my output rows = 1..32.
"""
import numpy as np

import concourse.bass as bass
import concourse.tile as tile
from concourse import bacc, mybir
from concourse.bass_utils import run_bass_kernel_spmd

F32 = mybir.dt.float32
F32R = mybir.dt.float32r
F16 = mybir.dt.float16
AF = mybir.ActivationFunctionType
ALU = mybir.AluOpType

B, CIN, H, W = 4, 256, 64, 64
CI, CQ, CO = 64, 8, 256
NCORES = 8
LR = 66                  # local feat1 rows
NP = LR * W              # 4224
NJT = NP // 128          # 33 j-tiles
WIN = 34 * W             # 2176
MY = 32 * W              # 2048
XR, XC = 68, 66          # x_pad rows/cols
NTAPS = 18               # 9 taps x 2 cin blocks
# i chunks: CAM uses full window; PAM main loop uses ICM + bf16 tail
IC = [(0, 512), (512, 512), (1024, 512), (1536, 512), (2048, 128)]
ICM = [(0, 512), (512, 512), (1024, 512), (1536, 384), (1920, 256)]
# conv1 output tiles: (row0, nrows)
C1T = [(8 * T, 8) for T in range(8)] + [(64, 2)]
C1GRP = [(0, 1), (2, 3), (4, 5), (6, 7, 8)]
N_STAT = 16384.0


# ---------------------------------------------------------------- host prep
def _rot_centers(h):
    P = [-1] + list(range(64)) + [64]
    return [P[(L + 32 * h) % 66] for L in range(LR)]


def _prep_core_inputs(x, w1, bn_g, bn_b, wq, bq, wk, bk, wv, bv,
                      gamma_pam, gamma_cam, w2, w8, b8):
    f = np.float32
    # shared weights
    w1s = np.zeros((128, NTAPS, CI), np.float16)
    for dy in range(3):
        for dx in range(3):
            for cb in range(2):
                s = (dy * 3 + dx) * 2 + cb
                w1s[:, s, :] = w1[:, cb * 128:(cb + 1) * 128, dy, dx].T
    wqkv = np.zeros((65, 80), f)
    wqkv[:64, 0:64] = wv[:, :, 0, 0].T
    wqkv[:64, 64:72] = wq[:, :, 0, 0].T
    wqkv[:64, 72:80] = wk[:, :, 0, 0].T
    wqkv[64, 0:64] = bv
    wqkv[64, 64:72] = bq
    wqkv[64, 72:80] = bk
    w2a = np.zeros((128, 3, CI), f)
    w2b = np.zeros((64, 3, CI), f)
    for dx in range(3):
        w2a[:64, dx, :] = w2[:, :, 0, dx].T
        w2a[64:, dx, :] = w2[:, :, 1, dx].T
        w2b[:, dx, :] = w2[:, :, 2, dx].T
    bngb = np.stack([bn_g, bn_b], 1).astype(f)
    consts = np.array([[float(gamma_pam[0]), float(gamma_cam[0])]], f)
    iden = np.eye(128, dtype=f)

    shared = dict(w1s=w1s, wqkv=wqkv, w2a=w2a.reshape(128, 3 * CI),
                  w2b=w2b.reshape(64, 3 * CI),
                  bngb=bngb, consts=consts, iden=iden)

    x16 = np.asarray(x, np.float16).reshape(B, 2, 128, H, W)
    in_maps = []
    for c in range(NCORES):
        b, h = divmod(c, 2)
        # my 32 canonical rows: [128, 2, 32, 64] fp16
        xh = np.ascontiguousarray(
            x16[b, :, :, 32 * h:32 * h + 32, :].transpose(1, 0, 2, 3))
        xsel = np.zeros((128, 2), np.float32)
        xsel[:, 0] = 1.0 if h == 0 else 0.0
        xsel[:, 1] = 1.0 - xsel[0, 0]
        centers = _rot_centers(h)
        real = np.array([0 <= g <= 63 for g in centers])
        realp = np.repeat(real, W)                        # [4224]
        ebias = np.stack([np.where(realp, 0.0, -1000.0).astype(f),
                          np.ones(NP, f)])
        nmask = np.where(realp, 1.0, 0.0).astype(f).reshape(NJT, 128).T.copy()
        hmask = np.zeros((64, 2), f)
        hmask[:, 0] = 0.0 if h == 0 else 1.0
        hmask[:, 1] = 0.0 if h == 1 else 1.0
        m = dict(shared)
        m.update(xh=xh, xsel=xsel, ebias=ebias, nmask=nmask, hmask=hmask)
        in_maps.append(m)
    return in_maps


# ---------------------------------------------------------------- bass build
def _build(nreps=1):
    nc = bacc.Bacc()
    xh = nc.declare_dram_parameter("xh", [128, 2, 32, 64], F16, isOutput=False)
    xsel = nc.declare_dram_parameter("xsel", [128, 2], F32, isOutput=False)
    w1s = nc.declare_dram_parameter("w1s", [128, NTAPS, CI], F16, isOutput=False)
    wqkv = nc.declare_dram_parameter("wqkv", [65, 80], F32R, isOutput=False)
    w2a = nc.declare_dram_parameter("w2a", [128, 3 * CI], F32R, isOutput=False)
    w2b = nc.declare_dram_parameter("w2b", [64, 3 * CI], F32R, isOutput=False)
    bngb = nc.declare_dram_parameter("bngb", [64, 2], F32, isOutput=False)
    ebias = nc.declare_dram_parameter("ebias", [2, NP], F32R, isOutput=False)
    nmask = nc.declare_dram_parameter("nmask", [128, NJT], F32, isOutput=False)
    hmask = nc.declare_dram_parameter("hmask", [64, 2], F32, isOutput=False)
    consts = nc.declare_dram_parameter("consts", [1, 2], F32, isOutput=False)
    iden = nc.declare_dram_parameter("iden", [128, 128], F32R, isOutput=False)
    out = nc.declare_dram_parameter("out", [64, MY], F16, isOutput=True)

    with tile.TileContext(nc) as tc:
        with tc.tile_pool(name="big", bufs=1) as big, \
             tc.tile_pool(name="xg", bufs=1) as xg, \
             tc.tile_pool(name="wt", bufs=1) as wt, \
             tc.tile_pool(name="sm", bufs=2) as sm, \
             tc.tile_pool(name="et", bufs=2) as etp, \
             tc.tile_pool(name="ps", bufs=2, space="PSUM") as ps, \
             tc.tile_pool(name="pt", bufs=2, space="PSUM") as ptp, \
             tc.tile_pool(name="mc", bufs=2, space="PSUM") as mcp, \
             tc.tile_pool(name="dram", bufs=1, space="DRAM") as dram:

            # ---- persistent sbuf tensors
            feat = big.tile([65, NP], F32R, tag="feat")   # y1 then feat1(+ones)
            qkv = big.tile([80, NP], F32R, tag="qkv")
            qr = big.tile([128, WIN], F32R, tag="qr")
            kr4 = big.tile([128, 9, 128], F32R, tag="kr4")
            vT = big.tile([128, NJT, 65], F32R, tag="vT")
            fT = big.tile([128, NJT, CI], F32R, tag="fT")
            sabuf = big.tile([128, 34, XC], F32R, tag="sabuf")
            scbuf = big.tile([128, 34, XC], F32R, tag="scbuf")
            y2a = big.tile([64, MY], F32, tag="y2a")
            y2b = big.tile([64, MY], F32, tag="y2b")
            pacc = big.tile([65, WIN], F32, tag="pacc")   # pam accumulator

            # ---- weights / consts in sbuf
            w1t = wt.tile([128, NTAPS, CI], F16, tag="w1t")
            wqkvt = wt.tile([65, 80], F32R, tag="wqkvt")
            w2at = wt.tile([128, 3 * CI], F32R, tag="w2at")
            w2bt = wt.tile([64, 3 * CI], F32R, tag="w2bt")
            bngbt = wt.tile([64, 2], F32, tag="bngbt")
            nmt = wt.tile([128, NJT], F32, tag="nmt")
            hmt = wt.tile([64, 2], F32, tag="hmt")
            xselt = wt.tile([128, 2], F32, tag="xselt")
            cst = wt.tile([1, 2], F32, tag="cst")
            gcam = wt.tile([64, 1], F32, tag="gcam")
            epst = wt.tile([64, 1], F32, tag="epst")
            nc.vector.memset(epst, 1e-5)
            idt = wt.tile([128, 128], F32R, tag="idt")
            nc.sync.dma_start(out=w1t, in_=w1s[:, :, :])
            nc.sync.dma_start(out=wqkvt, in_=wqkv[:, :])
            nc.sync.dma_start(out=w2at, in_=w2a[:, :])
            nc.sync.dma_start(out=w2bt, in_=w2b[:, :])
            nc.sync.dma_start(out=bngbt, in_=bngb[:, :])
            nc.sync.dma_start(out=nmt, in_=nmask[:, :])
            nc.sync.dma_start(out=hmt, in_=hmask[:, :])
            nc.sync.dma_start(out=xselt, in_=xsel[:, :])
            nc.sync.dma_start(out=cst, in_=consts[:, :])
            nc.sync.dma_start(out=idt, in_=iden[:, :])
            gc_src = bass.AP(tensor=consts, offset=1, ap=[[0, 64], [1, 1]])
            nc.gpsimd.dma_start(out=gcam, in_=gc_src)
            nc.gpsimd.memset(feat[64:65, :].bitcast(F32), 1.0)
            nc.gpsimd.memset(kr4[:, :, :].bitcast(F32), 0.0)
            nc.gpsimd.memset(vT[:, :, 64:65].bitcast(F32), 1.0)
            for bf in (sabuf, scbuf):
                nc.gpsimd.memset(bf[0:64, :, 0:1].bitcast(F32), 0.0)
                nc.gpsimd.memset(bf[0:64, :, 65:66].bitcast(F32), 0.0)

            def _body(rep):
                # ---- gather the full sample from my pair partner
                xmy = xg.tile([128, 2, 32, 64], F16, tag="xmy",
                              name=f"xmy{rep}")
                nc.sync.dma_start(out=xmy, in_=xh[:, :, :, :])
                xin_d = dram.tile([128, 2, 32, 64], F16, tag="xin_d",
                                  name=f"xin_d{rep}")
                nc.sync.dma_start(out=xin_d[:, :, :, :], in_=xmy)
                gx = dram.tile([256, 2, 32, 64], F16, tag="gx",
                               name=f"gx{rep}")
                nc.gpsimd.collective_compute(
                    "AllGather", ALU.bypass,
                    replica_groups=[[0, 1], [2, 3], [4, 5], [6, 7]],
                    ins=[xin_d[:, :, :, :].opt()],
                    outs=[gx[:, :, :, :].opt()])

                # ---- assemble both rotation patterns, mask-select mine.
                # pattern A (h=0): rows 2..65 <- canonical 0..63
                # pattern B (h=1): rows 0..1 <- 30..31, 2..33 <- 32..63,
                #                  36..67 <- 0..31
                xpA = xg.tile([128, 2, XR, XC], F16, tag="xpA",
                              name=f"xpA{rep}")
                xpB = xg.tile([128, 2, XR, XC], F16, tag="xpB",
                              name=f"xpB{rep}")
                nc.gpsimd.memset(xpA[:, :, :, :].bitcast(F32), 0.0)
                nc.gpsimd.memset(xpB[:, :, :, :].bitcast(F32), 0.0)
                for cb in range(2):
                    nc.sync.dma_start(out=xpA[:, cb, 2:34, 1:65],
                                      in_=gx[0:128, cb, 0:32, :])
                    nc.sync.dma_start(out=xpA[:, cb, 34:66, 1:65],
                                      in_=gx[128:256, cb, 0:32, :])
                    nc.sync.dma_start(out=xpB[:, cb, 0:2, 1:65],
                                      in_=gx[0:128, cb, 30:32, :])
                    nc.sync.dma_start(out=xpB[:, cb, 2:34, 1:65],
                                      in_=gx[128:256, cb, 0:32, :])
                    nc.sync.dma_start(out=xpB[:, cb, 36:68, 1:65],
                                      in_=gx[0:128, cb, 0:32, :])
                for cb in range(2):
                    nc.vector.tensor_scalar_mul(xpA[:, cb, :, :],
                                                xpA[:, cb, :, :],
                                                xselt[:, 0:1])
                    nc.vector.tensor_scalar_mul(xpB[:, cb, :, :],
                                                xpB[:, cb, :, :],
                                                xselt[:, 1:2])
                    nc.vector.tensor_tensor(xpA[:, cb, :, :],
                                            xpA[:, cb, :, :],
                                            xpB[:, cb, :, :], ALU.add)

                # ---- conv1 -> feat rows 0..63 hold raw y1
                stats1 = sm.tile([64, 5, 6], F32, tag="stats1")
                stat_slices = [(0, 64, 448), (1, 0, 512), (2, 0, 512),
                               (3, 0, 512), (4, 0, 64)]
                for grp in C1GRP:
                    pst = {}
                    for T in grp:
                        r0, nr = C1T[T]
                        pst[T] = mcp.tile([64, nr * W], F32, tag="mc",
                                          name=f"c1ps{T}")
                    for s in range(NTAPS):
                        tap, cb = divmod(s, 2)
                        dy, dx = divmod(tap, 3)
                        for T in grp:
                            r0, nr = C1T[T]
                            rhs = xpA[:, cb, r0 + dy:r0 + dy + nr, dx:dx + 64]
                            nc.tensor.matmul(pst[T], w1t[:, s, :], rhs,
                                             start=(s == 0), stop=(s == NTAPS - 1))
                    for T in grp:
                        r0, nr = C1T[T]
                        nc.vector.tensor_copy(feat[0:64, r0 * W:(r0 + nr) * W],
                                              pst[T])
                for (k, off, ln) in stat_slices:
                    T0 = [0, 512, 1024, 1536, 2048][k]
                    nc.vector.bn_stats(stats1[:, k, :],
                                       feat[0:64, T0 + off:T0 + off + ln])
                mv1 = sm.tile([64, 2], F32, tag="mv1")
                nc.vector.bn_aggr(mv1, stats1[:, :, :])

                def bn_coeffs(gl, tag):
                    """gl [64,2] = (sum, sumsq) -> (scale, shift) [64,1] f32."""
                    mean = sm.tile([64, 1], F32, tag=tag + "m", name=tag + "m")
                    var = sm.tile([64, 1], F32, tag=tag + "v", name=tag + "v")
                    scl = sm.tile([64, 1], F32, tag=tag + "s", name=tag + "s")
                    sh = sm.tile([64, 1], F32, tag=tag + "h", name=tag + "h")
                    nc.vector.tensor_scalar_mul(mean, gl[:, 0:1], 1.0 / N_STAT)
                    nc.vector.tensor_scalar_mul(var, gl[:, 1:2], 1.0 / N_STAT)
                    nc.vector.tensor_tensor(scl, mean, mean, ALU.mult)
                    nc.vector.tensor_tensor(var, var, scl, ALU.subtract)
                    nc.scalar.activation(var, var, AF.Sqrt, bias=epst, scale=1.0)
                    nc.vector.reciprocal(var, var)
                    nc.vector.tensor_tensor(scl, bngbt[:, 0:1], var, ALU.mult)
                    nc.vector.tensor_tensor(sh, mean, scl, ALU.mult)
                    nc.vector.tensor_tensor(sh, bngbt[:, 1:2], sh, ALU.subtract)
                    return scl, sh

                def stat_ar(mv, tag):
                    """partial (mean,var over MY) -> AllReduce -> (sum,sumsq)."""
                    ars = sm.tile([64, 2], F32, tag=tag + "s", name=tag + "s")
                    t_t = sm.tile([64, 1], F32, tag=tag + "t", name=tag + "t")
                    nc.vector.tensor_scalar_mul(ars[:, 0:1], mv[:, 0:1], float(MY))
                    nc.vector.tensor_tensor(t_t, mv[:, 0:1], mv[:, 0:1], ALU.mult)
                    nc.vector.tensor_tensor(t_t, mv[:, 1:2], t_t, ALU.add)
                    nc.vector.tensor_scalar_mul(ars[:, 1:2], t_t, float(MY))
                    a_in = dram.tile([64, 2], F32, tag=tag + "_in",
                                     name=tag + "_in")
                    a_out = dram.tile([64, 2], F32, tag=tag + "_out",
                                      name=tag + "_out")
                    nc.sync.dma_start(out=a_in[:, :], in_=ars)
                    nc.gpsimd.collective_compute(
                        "AllReduce", ALU.add,
                        replica_groups=[list(range(NCORES))],
                        ins=[a_in.opt()], outs=[a_out.opt()])
                    gl = sm.tile([64, 2], F32, tag=tag + "g", name=tag + "g")
                    nc.sync.dma_start(out=gl, in_=a_out[:, :])
                    return gl

                # AR1: bn1 stats
                gl1 = stat_ar(mv1, "ar1")
                sc1, sh1 = bn_coeffs(gl1, "bn1")
                for (r0, nr) in C1T:
                    sl = feat[0:64, r0 * W:(r0 + nr) * W]
                    nc.scalar.activation(sl, sl, AF.Relu, bias=sh1, scale=sc1)

                # ---- qkv
                qkvtiles = [(t * 512, 512) for t in range(8)] + [(4096, 128)]
                for ti, (c0, cw) in enumerate(qkvtiles):
                    qps = mcp.tile([80, cw], F32, tag="mc", name="qps")
                    nc.tensor.matmul(qps, wqkvt, feat[:, c0:c0 + cw],
                                     start=True, stop=True)
                    nc.vector.tensor_copy(qkv[:, c0:c0 + cw], qps)
                # qr: q replicated at partition groups; row 32g+8 = ones
                # (pairs with the ebias row in kr4 -> energy gets +ebias[j])
                for g in range(4):
                    nc.sync.dma_start(out=qr[32 * g:32 * g + 8, :],
                                      in_=qkv[64:72, 0:WIN])
                for g in range(4):
                    nc.sync.dma_start(out=qr[32 * g + 8:32 * g + 9, :],
                                      in_=ebias[1:2, 0:WIN])
                # kr4: k repartitioned per j-group; row 8 of each 32-block holds
                # the exp masking bias for that j-tile
                kbounce = dram.tile([8, NP], F32R, tag="kbounce", name="kbounce")
                nc.sync.dma_start(out=kbounce[:, :], in_=qkv[72:80, :])
                for u in range(4):
                    ksrc = bass.AP(tensor=kbounce.tensor,
                                   offset=kbounce.offset + u * 128,
                                   ap=[[NP, 8], [512, 8], [1, 128]])
                    nc.sync.dma_start(out=kr4[32 * u:32 * u + 8, 0:8, :],
                                      in_=ksrc)
                    bsrc = bass.AP(tensor=ebias, offset=u * 128,
                                   ap=[[512, 8], [1, 128]])
                    nc.sync.dma_start(out=kr4[32 * u + 8:32 * u + 9, 0:8, :],
                                      in_=bsrc)
                nc.sync.dma_start(out=kr4[0:8, 8, :], in_=kbounce[:, 4096:4224])
                nc.sync.dma_start(out=kr4[8:9, 8, :], in_=ebias[0:1, 4096:4224])

                # ---- vT transpose (+ones col), 4 per psum bank
                for j0 in range(0, 32, 4):
                    tp = mcp.tile([128, 4, 64], F32R, tag="mc",
                                  name=f"vtp{j0}")
                    for k in range(4):
                        jt = j0 + k
                        nc.tensor.transpose(
                            tp[:, k, :],
                            qkv[0:64, jt * 128:(jt + 1) * 128],
                            idt[0:64, 0:64])
                    nc.vector.tensor_copy(vT[:, j0:j0 + 4, 0:64], tp)
                tpl = mcp.tile([128, 64], F32R, tag="mc", name="vtpl")
                nc.tensor.transpose(tpl, qkv[0:64, 32 * 128:33 * 128],
                                    idt[0:64, 0:64])
                nc.vector.tensor_copy(vT[:, 32, 0:64], tpl)

                # ================= interleaved attention + CAM emission ========
                def pam_pair(jg0, chunk_cb=None):
                    """Emit energy/exp/pam for j-groups jg0, jg0+1 (or lone 8)."""
                    jgs = [jg0] if jg0 == 8 else [jg0, jg0 + 1]
                    for ici, (i0, iw) in enumerate(ICM):
                        pt = ptp.tile([65, iw], F32, tag="pt", name="pt")
                        nmm = sum(4 if j < 8 else 1 for j in jgs)
                        k = 0
                        for jg in jgs:
                            nu2 = 2 if jg < 8 else 1
                            for p in range(2 if jg < 8 else 1):
                                et_ps = ps.tile([128, 2, 512], F32, tag="ps",
                                                name="et_ps")
                                for u2 in range(nu2):
                                    u = 2 * p + u2
                                    nc.tensor.matmul(
                                        et_ps[:, u2, 0:iw],
                                        kr4[32 * u:32 * u + 32, jg, :],
                                        qr[32 * u:32 * u + 32, i0:i0 + iw],
                                        start=True, stop=True,
                                        tile_position=(32 * u, 0))
                                eT = etp.tile([128, 2, 512], F32R, tag="et",
                                              bufs=2, name="eT")
                                if nu2 == 2:
                                    nc.scalar.activation(eT[:, :, 0:iw],
                                                         et_ps[:, :, 0:iw],
                                                         AF.Exp, bias=0.0,
                                                         scale=1.0)
                                else:
                                    nc.scalar.activation(eT[:, 0, 0:iw],
                                                         et_ps[:, 0, 0:iw],
                                                         AF.Exp, bias=0.0,
                                                         scale=1.0)
                                for u2 in range(nu2):
                                    jt = 4 * jg + 2 * p + u2
                                    nc.tensor.matmul(pt, vT[:, jt, :],
                                                     eT[:, u2, 0:iw],
                                                     start=(k == 0),
                                                     stop=(k == nmm - 1))
                                    k += 1
                        if jg0 == 0:
                            nc.vector.tensor_copy(pacc[:, i0:i0 + iw], pt)
                        else:
                            nc.vector.tensor_tensor(pacc[:, i0:i0 + iw],
                                                    pacc[:, i0:i0 + iw], pt,
                                                    ALU.add)
                        if chunk_cb is not None:
                            chunk_cb(ici, i0, iw)

                pam_pair(0)
                # fT transposes (CAM input), masked
                for jt in range(NJT):
                    tp = mcp.tile([128, 64], F32R, tag="mc", name=f"ftp{jt}")
                    nc.tensor.transpose(tp, feat[0:64, jt * 128:(jt + 1) * 128],
                                        idt[0:64, 0:64])
                    nc.vector.tensor_scalar_mul(fT[:, jt, :], tp, nmt[:, jt:jt + 1])

                pam_pair(2)
                # CAM: ce (chunked), softmax, cattnT
                ce_sb = sm.tile([64, 64], F32, tag="ce_sb")
                for ci_, (j0, nj) in enumerate([(0, 9), (9, 8), (17, 8), (25, 8)]):
                    ce_ps = mcp.tile([64, 64], F32, tag="mc", name=f"ce{ci_}")
                    for k in range(nj):
                        jt = j0 + k
                        nc.tensor.matmul(ce_ps, fT[:, jt, :], fT[:, jt, :],
                                         start=(k == 0), stop=(k == nj - 1))
                    if ci_ == 0:
                        nc.vector.tensor_copy(ce_sb, ce_ps)
                    else:
                        nc.vector.tensor_tensor(ce_sb, ce_sb, ce_ps, ALU.add)
                rmin = sm.tile([64, 1], F32, tag="rmin")
                nc.vector.tensor_reduce(rmin, ce_sb, mybir.AxisListType.X, ALU.min)
                cu = sm.tile([64, 64], F32, tag="cu")
                nc.scalar.activation(cu, ce_sb, AF.Exp, bias=rmin, scale=-1.0)
                rs = sm.tile([64, 1], F32, tag="rs")
                nc.vector.tensor_reduce(rs, cu, mybir.AxisListType.X, ALU.add)
                nc.vector.reciprocal(rs, rs)
                cattn = sm.tile([64, 64], F32R, tag="cattn")
                nc.vector.tensor_scalar_mul(cattn, cu, rs)
                ctp = mcp.tile([64, 64], F32R, tag="mc", name="ctp")
                nc.tensor.transpose(ctp, cattn, idt[0:64, 0:64])
                cattnT = sm.tile([64, 64], F32R, tag="cattnT")
                nc.vector.tensor_copy(cattnT, ctp)

                pam_pair(4)
                # CAM apply + scbuf
                for (i0, iw) in IC:
                    cam_ps = mcp.tile([64, iw], F32, tag="mc", name="cam_ps")
                    nc.tensor.matmul(cam_ps, cattnT, feat[0:64, i0:i0 + iw],
                                     start=True, stop=True)
                    tmpc = etp.tile([64, iw], F32R, tag="camt", bufs=3,
                                    name="tmpc")
                    nc.vector.tensor_scalar_mul(tmpc, cam_ps, gcam)
                    r0, nr = i0 // W, iw // W
                    nc.vector.tensor_tensor(
                        scbuf[0:64, r0:r0 + nr, 1:65],
                        tmpc[:, :].rearrange("p (r c) -> p r c", c=W),
                        feat[0:64, i0:i0 + iw].rearrange("p (r c) -> p r c", c=W),
                        ALU.add)
                nc.vector.tensor_scalar_mul(scbuf[0:64, 0, 1:65],
                                            scbuf[0:64, 0, 1:65], hmt[:, 0:1])
                nc.vector.tensor_scalar_mul(scbuf[0:64, 33, 1:65],
                                            scbuf[0:64, 33, 1:65], hmt[:, 1:2])
                for (a, b) in [(0, 9), (9, 17), (17, 25), (25, 33)]:
                    nc.gpsimd.tensor_copy(scbuf[64:128, a:b, :],
                                          scbuf[0:64, a + 1:b + 1, :])

                def conv2(buf, y2sb, sttag):
                    st = sm.tile([64, 4, 6], F32, tag=sttag, name=sttag)
                    for T in range(4):
                        r0 = 1 + 8 * T
                        yps = mcp.tile([64, 512], F32, tag="mc", name="yps")
                        for dxi in range(3):
                            rhs1 = buf[:, r0 - 1:r0 + 7, dxi:dxi + 64]
                            nc.tensor.matmul(yps, w2at[:, dxi * 64:(dxi + 1) * 64],
                                             rhs1, start=(dxi == 0), stop=False)
                            rhs2 = buf[0:64, r0 + 1:r0 + 9, dxi:dxi + 64]
                            nc.tensor.matmul(yps, w2bt[:, dxi * 64:(dxi + 1) * 64],
                                             rhs2, start=False, stop=(dxi == 2))
                        nc.vector.bn_stats(st[:, T, :], yps)
                        nc.vector.tensor_copy(y2sb[:, T * 512:(T + 1) * 512], yps)
                    mv = sm.tile([64, 2], F32, tag=sttag + "mv", name=sttag + "mv")
                    nc.vector.bn_aggr(mv, st[:, :, :])
                    return mv

                pam_pair(6)
                # conv2 on CAM branch + its stats AR (hidden under attention)
                mvb = conv2(scbuf, y2b, "stb")
                glb = stat_ar(mvb, "arb")
                scb, shb = bn_coeffs(glb, "bnb")
                rb = big.tile([64, MY], F32R, tag="rb")
                nc.scalar.activation(rb, y2b, AF.Relu, bias=shb, scale=scb)

                # ---- pam normalize (r = gamma_pam / s), sa = pam_u*r + feat1
                def pam_div(src, i0, iw, sfx):
                    r32 = sm.tile([1, iw], F32, tag="r32", name="r32" + sfx)
                    nc.vector.reciprocal(r32, src[64:65, :])
                    rr = sm.tile([1, iw], F32R, tag="rr", name="rr" + sfx)
                    nc.vector.tensor_scalar_mul(rr, r32, cst[0:1, 0:1])
                    rbc = etp.tile([64, iw], F32R, tag="camt", bufs=3,
                                   name="rbc" + sfx)
                    nc.gpsimd.partition_broadcast(rbc, rr)
                    tmpa = etp.tile([64, iw], F32R, tag="camt", bufs=3,
                                    name="tmpa" + sfx)
                    nc.vector.tensor_tensor(tmpa, src[0:64, :], rbc, ALU.mult)
                    r0, nr = i0 // W, iw // W
                    nc.vector.tensor_tensor(
                        sabuf[0:64, r0:r0 + nr, 1:65],
                        tmpa[:, :].rearrange("p (r c) -> p r c", c=W),
                        feat[0:64, i0:i0 + iw].rearrange("p (r c) -> p r c", c=W),
                        ALU.add)

                pam_pair(8, chunk_cb=lambda ici, i0, iw: pam_div(
                    pacc[:, i0:i0 + iw], i0, iw, str(ici)))
                nc.vector.tensor_scalar_mul(sabuf[0:64, 0, 1:65],
                                            sabuf[0:64, 0, 1:65], hmt[:, 0:1])
                nc.vector.tensor_scalar_mul(sabuf[0:64, 33, 1:65],
                                            sabuf[0:64, 33, 1:65], hmt[:, 1:2])
                for (a, b) in [(0, 9), (9, 17), (17, 25), (25, 33)]:
                    nc.gpsimd.tensor_copy(sabuf[64:128, a:b, :],
                                          sabuf[0:64, a + 1:b + 1, :])

                mva = conv2(sabuf, y2a, "sta")
                gla = stat_ar(mva, "ara")
                sca, sha = bn_coeffs(gla, "bna")

                # ---- relu + sum -> fp16 feat_sum out (conv8 runs on host)
                for T in range(4):
                    sl = slice(T * 512, (T + 1) * 512)
                    ra = etp.tile([64, 512], F32R, tag="camt", bufs=3,
                                  name=f"ra{T}")
                    nc.scalar.activation(ra, y2a[:, sl], AF.Relu,
                                         bias=sha, scale=sca)
                    fo = etp.tile([64, 512], F16, tag="fo", bufs=3,
                                  name=f"fo{T}")
                    nc.vector.tensor_tensor(fo, ra, rb[:, sl], ALU.add)
                    nc.sync.dma_start(out=out[:, sl], in_=fo)

            for rep in range(nreps):
                _body(rep)
    nc.finalize()
    return nc


_NC_CACHE = {}


def kernel(**inputs):
    if "nc" not in _NC_CACHE:
        _NC_CACHE["nc"] = _build()
    nc = _NC_CACHE["nc"]
    x = np.asarray(inputs["x"], np.float32)
    w8 = np.asarray(inputs["w8"], np.float32)
    b8 = np.asarray(inputs["b8"], np.float32)
    in_maps = _prep_core_inputs(
        x, np.asarray(inputs["w1"]), np.asarray(inputs["bn_g"]),
        np.asarray(inputs["bn_b"]), np.asarray(inputs["wq"]),
        np.asarray(inputs["bq"]), np.asarray(inputs["wk"]),
        np.asarray(inputs["bk"]), np.asarray(inputs["wv"]),
        np.asarray(inputs["bv"]), np.asarray(inputs["gamma_pam"]),
        np.asarray(inputs["gamma_cam"]), np.asarray(inputs["w2"]),
        w8, b8)
    res = run_bass_kernel_spmd(nc, in_maps, list(range(NCORES)))
    fs = np.empty((B, CI, H * W), np.float32)
    for c in range(NCORES):
        b, h = divmod(c, 2)
        fs[b, :, MY * h:MY * h + MY] = res.results[c]["out"]
    W8 = w8[:, :, 0, 0]
    out = np.matmul(W8[None], fs) + b8[None, :, None]
    return out.reshape(B, CO, H, W).astype(np.float32)


# revision 8
# speedup vs baseline: 17.1702x; 1.7687x over previous
"""DANetHead Trainium2 kernel: 8-core SPMD (batch x row-half sharding).

Self-contained: hardcodes all shapes from the problem spec.

I/O-optimized vs the v1 baseline (wall time through the axon tunnel is
dominated by host<->device bytes, not device compute):
  * x is exact-sharded: each core uploads only its 32 rows of its sample
    in fp16 ([128, 2, 32, 64], 1.05 MB); the full sample is rebuilt on
    device with a pair AllGather.
  * The h-dependent row rotation (the per-core attention window trick)
    cannot pass through the shared AllGather (both pair cores run the
    same code on the same gathered bytes), so BOTH rotation patterns are
    assembled in SBUF and blended with a per-core 0/1 mask (xsel).
  * conv1 runs in fp16 (weights uploaded fp16).
  * The final 1x1 conv8 (64->256 ch) is moved to the host: the device
    returns the 64-channel feat_sum in fp16 (0.26 MB/core instead of
    2 MB/core), and numpy applies W8 @ feat_sum + b8.

Per-core layout (core c: sample b=c//2, half h=c%2):
  P = [-1, 0..63, 64] (66 padded rows; -1/64 zero).
  x_pad rows R=0..67 hold padded row P[(R-1+32h) % 66]  (cyclic rotation, so
  every core's attention/conv2 window is local rows 0..33 uniformly).
  conv1 output local row L (0..65) centers on P[(L+32h) % 66].
  window = local rows 0..33 (flat 0..2175); my output rows = 1..32.
"""
import os
import tempfile

import numpy as np

try:  # persistent XLA compile cache: skips ~0.4s/call of re-lowering
    import jax
    jax.config.update("jax_compilation_cache_dir",
                      os.path.join(tempfile.gettempdir(), "danet_jax_cache"))
    jax.config.update("jax_persistent_cache_min_compile_time_secs", 0)
    jax.config.update("jax_persistent_cache_min_entry_size_bytes", -1)
except Exception:
    pass

import concourse.bass as bass
import concourse.tile as tile
from concourse import bacc, mybir
from concourse.bass_utils import run_bass_kernel_spmd

F32 = mybir.dt.float32
F32R = mybir.dt.float32r
F16 = mybir.dt.float16
AF = mybir.ActivationFunctionType
ALU = mybir.AluOpType

B, CIN, H, W = 4, 256, 64, 64
CI, CQ, CO = 64, 8, 256
NCORES = 8
LR = 66                  # local feat1 rows
NP = LR * W              # 4224
NJT = NP // 128          # 33 j-tiles
WIN = 34 * W             # 2176
MY = 32 * W              # 2048
XR, XC = 68, 66          # x_pad rows/cols
NTAPS = 18               # 9 taps x 2 cin blocks
# i chunks: CAM uses full window; PAM main loop uses ICM + bf16 tail
IC = [(0, 512), (512, 512), (1024, 512), (1536, 512), (2048, 128)]
ICM = [(0, 512), (512, 512), (1024, 512), (1536, 384), (1920, 256)]
# conv1 output tiles: (row0, nrows)
C1T = [(8 * T, 8) for T in range(8)] + [(64, 2)]
C1GRP = [(0, 1), (2, 3), (4, 5), (6, 7, 8)]
N_STAT = 16384.0

# shared-weight fp16 mega-blob (f32 tensors stored as raw byte pairs),
# sharded 8 ways and AllGathered on device. Offsets in fp16 elements.
W_W1 = 0                      # w1s  f16 [128, 1152]
W_QK = W_W1 + 128 * 1152      # wqkv f32 [65, 80]  -> 65x160 f16
W_2A = W_QK + 65 * 160        # w2a  f32 [128, 192] -> 128x384
W_2B = W_2A + 128 * 384       # w2b  f32 [64, 192]  -> 64x384
W_ID = W_2B + 64 * 384        # iden f32 [128, 128] -> 128x256
W_BG = W_ID + 128 * 256       # bngb f32 [64, 2]    -> 64x4
W_CS = W_BG + 64 * 4          # consts f32 [1, 2]   -> 1x4
W_TOT = W_CS + 4
W_SH = -(-W_TOT // 16) * 2    # per-core shard length (pad to mult of 8)
W_PAD = W_SH * 8

# per-core aux blob (f32 elements)
A_EB = 0                      # ebias [2, 4224]
A_NM = A_EB + 2 * NP          # nmask [128, 33]
A_HM = A_NM + 128 * NJT       # hmask [64, 2]
A_XS = A_HM + 64 * 2          # xsel  [128, 2]
A_TOT = A_XS + 128 * 2


# ---------------------------------------------------------------- host prep
def _rot_centers(h):
    P = [-1] + list(range(64)) + [64]
    return [P[(L + 32 * h) % 66] for L in range(LR)]


def _prep_core_inputs(x, w1, bn_g, bn_b, wq, bq, wk, bk, wv, bv,
                      gamma_pam, gamma_cam, w2, w8, b8):
    f = np.float32
    # shared weights -> one fp16 mega-blob (f32 data stored as raw bytes)
    blob = np.zeros(W_PAD, np.float16)

    def put16(off, arr16):
        a = arr16.ravel()
        blob[off:off + a.size] = a

    def put32(off, arr32):
        a = np.ascontiguousarray(arr32, f).ravel().view(np.float16)
        blob[off:off + a.size] = a

    w1s = np.zeros((128, NTAPS, CI), np.float16)
    for dy in range(3):
        for dx in range(3):
            for cb in range(2):
                s = (dy * 3 + dx) * 2 + cb
                w1s[:, s, :] = w1[:, cb * 128:(cb + 1) * 128, dy, dx].T
    put16(W_W1, w1s)
    wqkv = np.zeros((65, 80), f)
    wqkv[:64, 0:64] = wv[:, :, 0, 0].T
    wqkv[:64, 64:72] = wq[:, :, 0, 0].T
    wqkv[:64, 72:80] = wk[:, :, 0, 0].T
    wqkv[64, 0:64] = bv
    wqkv[64, 64:72] = bq
    wqkv[64, 72:80] = bk
    put32(W_QK, wqkv)
    w2a = np.zeros((128, 3, CI), f)
    w2b = np.zeros((64, 3, CI), f)
    for dx in range(3):
        w2a[:64, dx, :] = w2[:, :, 0, dx].T
        w2a[64:, dx, :] = w2[:, :, 1, dx].T
        w2b[:, dx, :] = w2[:, :, 2, dx].T
    put32(W_2A, w2a)
    put32(W_2B, w2b)
    put32(W_ID, np.eye(128, dtype=f))
    put32(W_BG, np.stack([bn_g, bn_b], 1).astype(f))
    put32(W_CS, np.array([[float(gamma_pam[0]), float(gamma_cam[0])]], f))
    blob = blob.reshape(8, 1, W_SH)

    x16 = np.asarray(x, np.float16).reshape(B, 2, 128, H, W)
    in_maps = []
    for c in range(NCORES):
        b, h = divmod(c, 2)
        # my 32 canonical rows: [128, 2, 32, 64] fp16
        xh = np.ascontiguousarray(
            x16[b, :, :, 32 * h:32 * h + 32, :].transpose(1, 0, 2, 3))
        aux = np.zeros(A_TOT, f)
        centers = _rot_centers(h)
        real = np.array([0 <= g <= 63 for g in centers])
        realp = np.repeat(real, W)                        # [4224]
        aux[A_EB:A_EB + NP] = np.where(realp, 0.0, -1000.0)
        aux[A_EB + NP:A_EB + 2 * NP] = 1.0
        aux[A_NM:A_NM + 128 * NJT] = \
            np.where(realp, 1.0, 0.0).astype(f).reshape(NJT, 128).T.ravel()
        hm = aux[A_HM:A_HM + 128].reshape(64, 2)
        hm[:, 0] = 0.0 if h == 0 else 1.0
        hm[:, 1] = 0.0 if h == 1 else 1.0
        xs = aux[A_XS:A_XS + 256].reshape(128, 2)
        xs[:, 0] = 1.0 if h == 0 else 0.0
        xs[:, 1] = 1.0 - xs[0, 0]
        in_maps.append(dict(xh=xh, wsh=blob[c], aux=aux.reshape(1, A_TOT)))
    return in_maps


# ---------------------------------------------------------------- bass build
def _build(nreps=1):
    nc = bacc.Bacc()
    xh = nc.declare_dram_parameter("xh", [128, 2, 32, 64], F16, isOutput=False)
    wsh = nc.declare_dram_parameter("wsh", [1, W_SH], F16, isOutput=False)
    aux = nc.declare_dram_parameter("aux", [1, A_TOT], F32R, isOutput=False)
    out = nc.declare_dram_parameter("out", [64, MY], F16, isOutput=True)

    with tile.TileContext(nc) as tc:
        with tc.tile_pool(name="big", bufs=1) as big, \
             tc.tile_pool(name="xg", bufs=1) as xg, \
             tc.tile_pool(name="wt", bufs=1) as wt, \
             tc.tile_pool(name="sm", bufs=2) as sm, \
             tc.tile_pool(name="et", bufs=2) as etp, \
             tc.tile_pool(name="ps", bufs=2, space="PSUM") as ps, \
             tc.tile_pool(name="pt", bufs=2, space="PSUM") as ptp, \
             tc.tile_pool(name="mc", bufs=2, space="PSUM") as mcp, \
             tc.tile_pool(name="dram", bufs=1, space="DRAM") as dram:

            # ---- persistent sbuf tensors
            feat = big.tile([65, NP], F32R, tag="feat")   # y1 then feat1(+ones)
            qkv = big.tile([80, NP], F32R, tag="qkv")
            qr = big.tile([128, WIN], F32R, tag="qr")
            kr4 = big.tile([128, 9, 128], F32R, tag="kr4")
            vT = big.tile([128, NJT, 65], F32R, tag="vT")
            fT = big.tile([128, NJT, CI], F32R, tag="fT")
            sabuf = big.tile([128, 34, XC], F32R, tag="sabuf")
            scbuf = big.tile([128, 34, XC], F32R, tag="scbuf")
            y2a = big.tile([64, MY], F32, tag="y2a")
            y2b = big.tile([64, MY], F32, tag="y2b")
            pacc = big.tile([65, WIN], F32, tag="pacc")   # pam accumulator

            # ---- gather the sharded shared-weight blob from all 8 cores
            win_d = dram.tile([1, W_SH], F16, tag="win_d")
            nc.sync.dma_start(out=win_d[:, :], in_=wsh[:, :])
            gwd = dram.tile([1, W_PAD], F16, tag="gwd")
            nc.gpsimd.collective_compute(
                "AllGather", ALU.bypass,
                replica_groups=[list(range(NCORES))],
                ins=[win_d[:, :].opt()], outs=[gwd[:, :].opt()])

            def wsrc(off, ap):
                return bass.AP(tensor=gwd.tensor, offset=gwd.offset + off,
                               ap=ap)

            # ---- weights / consts in sbuf
            w1t = wt.tile([128, NTAPS, CI], F16, tag="w1t")
            wqkvt = wt.tile([65, 80], F32R, tag="wqkvt")
            w2at = wt.tile([128, 3 * CI], F32R, tag="w2at")
            w2bt = wt.tile([64, 3 * CI], F32R, tag="w2bt")
            bngbt = wt.tile([64, 2], F32, tag="bngbt")
            nmt = wt.tile([128, NJT], F32, tag="nmt")
            hmt = wt.tile([64, 2], F32, tag="hmt")
            xselt = wt.tile([128, 2], F32, tag="xselt")
            cst = wt.tile([1, 2], F32, tag="cst")
            gcam = wt.tile([64, 1], F32, tag="gcam")
            epst = wt.tile([64, 1], F32, tag="epst")
            nc.vector.memset(epst, 1e-5)
            idt = wt.tile([128, 128], F32R, tag="idt")
            nc.sync.dma_start(out=w1t,
                              in_=wsrc(W_W1, [[1152, 128], [1, 1152]]))
            nc.sync.dma_start(out=wqkvt.bitcast(F16),
                              in_=wsrc(W_QK, [[160, 65], [1, 160]]))
            nc.sync.dma_start(out=w2at.bitcast(F16),
                              in_=wsrc(W_2A, [[384, 128], [1, 384]]))
            nc.sync.dma_start(out=w2bt.bitcast(F16),
                              in_=wsrc(W_2B, [[384, 64], [1, 384]]))
            nc.sync.dma_start(out=idt.bitcast(F16),
                              in_=wsrc(W_ID, [[256, 128], [1, 256]]))
            nc.sync.dma_start(out=bngbt.bitcast(F16),
                              in_=wsrc(W_BG, [[4, 64], [1, 4]]))
            nc.sync.dma_start(out=cst.bitcast(F16),
                              in_=wsrc(W_CS, [[4, 1], [1, 4]]))
            nc.sync.dma_start(out=nmt.bitcast(F32R),
                              in_=bass.AP(tensor=aux, offset=A_NM,
                                          ap=[[NJT, 128], [1, NJT]]))
            nc.sync.dma_start(out=hmt.bitcast(F32R),
                              in_=bass.AP(tensor=aux, offset=A_HM,
                                          ap=[[2, 64], [1, 2]]))
            nc.sync.dma_start(out=xselt.bitcast(F32R),
                              in_=bass.AP(tensor=aux, offset=A_XS,
                                          ap=[[2, 128], [1, 2]]))
            nc.gpsimd.partition_broadcast(gcam, cst[0:1, 1:2])
            nc.gpsimd.memset(feat[64:65, :].bitcast(F32), 1.0)
            nc.gpsimd.memset(kr4[:, :, :].bitcast(F32), 0.0)
            nc.gpsimd.memset(vT[:, :, 64:65].bitcast(F32), 1.0)
            for bf in (sabuf, scbuf):
                nc.gpsimd.memset(bf[0:64, :, 0:1].bitcast(F32), 0.0)
                nc.gpsimd.memset(bf[0:64, :, 65:66].bitcast(F32), 0.0)

            def _body(rep):
                # ---- gather the full sample from my pair partner
                xin_d = dram.tile([128, 2, 32, 64], F16, tag="xin_d",
                                  name=f"xin_d{rep}")
                nc.sync.dma_start(out=xin_d[:, :, :, :], in_=xh[:, :, :, :])
                gx = dram.tile([256, 2, 32, 64], F16, tag="gx",
                               name=f"gx{rep}")
                nc.gpsimd.collective_compute(
                    "AllGather", ALU.bypass,
                    replica_groups=[[0, 1], [2, 3], [4, 5], [6, 7]],
                    ins=[xin_d[:, :, :, :].opt()],
                    outs=[gx[:, :, :, :].opt()])

                # ---- assemble both rotation patterns, mask-select mine.
                # pattern A (h=0): rows 2..65 <- canonical 0..63
                # pattern B (h=1): rows 0..1 <- 30..31, 2..33 <- 32..63,
                #                  36..67 <- 0..31
                xpA = xg.tile([128, 2, XR, XC], F16, tag="xpA",
                              name=f"xpA{rep}")
                xpB = xg.tile([128, 2, XR, XC], F16, tag="xpB",
                              name=f"xpB{rep}")
                nc.gpsimd.memset(xpA[:, :, :, :].bitcast(F32), 0.0)
                nc.gpsimd.memset(xpB[:, :, :, :].bitcast(F32), 0.0)
                for cb in range(2):
                    nc.sync.dma_start(out=xpA[:, cb, 2:34, 1:65],
                                      in_=gx[0:128, cb, 0:32, :])
                    nc.sync.dma_start(out=xpA[:, cb, 34:66, 1:65],
                                      in_=gx[128:256, cb, 0:32, :])
                    nc.sync.dma_start(out=xpB[:, cb, 0:2, 1:65],
                                      in_=gx[0:128, cb, 30:32, :])
                    nc.sync.dma_start(out=xpB[:, cb, 2:34, 1:65],
                                      in_=gx[128:256, cb, 0:32, :])
                    nc.sync.dma_start(out=xpB[:, cb, 36:68, 1:65],
                                      in_=gx[0:128, cb, 0:32, :])
                for cb in range(2):
                    nc.vector.tensor_scalar_mul(xpA[:, cb, :, :],
                                                xpA[:, cb, :, :],
                                                xselt[:, 0:1])
                    nc.vector.tensor_scalar_mul(xpB[:, cb, :, :],
                                                xpB[:, cb, :, :],
                                                xselt[:, 1:2])
                    nc.vector.tensor_tensor(xpA[:, cb, :, :],
                                            xpA[:, cb, :, :],
                                            xpB[:, cb, :, :], ALU.add)

                # ---- conv1 -> feat rows 0..63 hold raw y1
                stats1 = sm.tile([64, 5, 6], F32, tag="stats1")
                stat_slices = [(0, 64, 448), (1, 0, 512), (2, 0, 512),
                               (3, 0, 512), (4, 0, 64)]
                for grp in C1GRP:
                    pst = {}
                    for T in grp:
                        r0, nr = C1T[T]
                        pst[T] = mcp.tile([64, nr * W], F32, tag="mc",
                                          name=f"c1ps{T}")
                    for s in range(NTAPS):
                        tap, cb = divmod(s, 2)
                        dy, dx = divmod(tap, 3)
                        for T in grp:
                            r0, nr = C1T[T]
                            rhs = xpA[:, cb, r0 + dy:r0 + dy + nr, dx:dx + 64]
                            nc.tensor.matmul(pst[T], w1t[:, s, :], rhs,
                                             start=(s == 0), stop=(s == NTAPS - 1))
                    for T in grp:
                        r0, nr = C1T[T]
                        nc.vector.tensor_copy(feat[0:64, r0 * W:(r0 + nr) * W],
                                              pst[T])
                for (k, off, ln) in stat_slices:
                    T0 = [0, 512, 1024, 1536, 2048][k]
                    nc.vector.bn_stats(stats1[:, k, :],
                                       feat[0:64, T0 + off:T0 + off + ln])
                mv1 = sm.tile([64, 2], F32, tag="mv1")
                nc.vector.bn_aggr(mv1, stats1[:, :, :])

                def bn_coeffs(gl, tag):
                    """gl [64,2] = (sum, sumsq) -> (scale, shift) [64,1] f32."""
                    mean = sm.tile([64, 1], F32, tag=tag + "m", name=tag + "m")
                    var = sm.tile([64, 1], F32, tag=tag + "v", name=tag + "v")
                    scl = sm.tile([64, 1], F32, tag=tag + "s", name=tag + "s")
                    sh = sm.tile([64, 1], F32, tag=tag + "h", name=tag + "h")
                    nc.vector.tensor_scalar_mul(mean, gl[:, 0:1], 1.0 / N_STAT)
                    nc.vector.tensor_scalar_mul(var, gl[:, 1:2], 1.0 / N_STAT)
                    nc.vector.tensor_tensor(scl, mean, mean, ALU.mult)
                    nc.vector.tensor_tensor(var, var, scl, ALU.subtract)
                    nc.scalar.activation(var, var, AF.Sqrt, bias=epst, scale=1.0)
                    nc.vector.reciprocal(var, var)
                    nc.vector.tensor_tensor(scl, bngbt[:, 0:1], var, ALU.mult)
                    nc.vector.tensor_tensor(sh, mean, scl, ALU.mult)
                    nc.vector.tensor_tensor(sh, bngbt[:, 1:2], sh, ALU.subtract)
                    return scl, sh

                def stat_ar(mv, tag):
                    """partial (mean,var over MY) -> AllReduce -> (sum,sumsq)."""
                    ars = sm.tile([64, 2], F32, tag=tag + "s", name=tag + "s")
                    t_t = sm.tile([64, 1], F32, tag=tag + "t", name=tag + "t")
                    nc.vector.tensor_scalar_mul(ars[:, 0:1], mv[:, 0:1], float(MY))
                    nc.vector.tensor_tensor(t_t, mv[:, 0:1], mv[:, 0:1], ALU.mult)
                    nc.vector.tensor_tensor(t_t, mv[:, 1:2], t_t, ALU.add)
                    nc.vector.tensor_scalar_mul(ars[:, 1:2], t_t, float(MY))
                    a_in = dram.tile([64, 2], F32, tag=tag + "_in",
                                     name=tag + "_in")
                    a_out = dram.tile([64, 2], F32, tag=tag + "_out",
                                      name=tag + "_out")
                    nc.sync.dma_start(out=a_in[:, :], in_=ars)
                    nc.gpsimd.collective_compute(
                        "AllReduce", ALU.add,
                        replica_groups=[list(range(NCORES))],
                        ins=[a_in.opt()], outs=[a_out.opt()])
                    gl = sm.tile([64, 2], F32, tag=tag + "g", name=tag + "g")
                    nc.sync.dma_start(out=gl, in_=a_out[:, :])
                    return gl

                # AR1: bn1 stats
                gl1 = stat_ar(mv1, "ar1")
                sc1, sh1 = bn_coeffs(gl1, "bn1")
                for (r0, nr) in C1T:
                    sl = feat[0:64, r0 * W:(r0 + nr) * W]
                    nc.scalar.activation(sl, sl, AF.Relu, bias=sh1, scale=sc1)

                # ---- qkv
                qkvtiles = [(t * 512, 512) for t in range(8)] + [(4096, 128)]
                for ti, (c0, cw) in enumerate(qkvtiles):
                    qps = mcp.tile([80, cw], F32, tag="mc", name="qps")
                    nc.tensor.matmul(qps, wqkvt, feat[:, c0:c0 + cw],
                                     start=True, stop=True)
                    nc.vector.tensor_copy(qkv[:, c0:c0 + cw], qps)
                # qr: q replicated at partition groups; row 32g+8 = ones
                # (pairs with the ebias row in kr4 -> energy gets +ebias[j])
                for g in range(4):
                    nc.sync.dma_start(out=qr[32 * g:32 * g + 8, :],
                                      in_=qkv[64:72, 0:WIN])
                for g in range(4):
                    nc.sync.dma_start(
                        out=qr[32 * g + 8:32 * g + 9, :],
                        in_=bass.AP(tensor=aux, offset=A_EB + NP,
                                    ap=[[NP, 1], [1, WIN]]))
                # kr4: k repartitioned per j-group; row 8 of each 32-block holds
                # the exp masking bias for that j-tile
                kbounce = dram.tile([8, NP], F32R, tag="kbounce", name="kbounce")
                nc.sync.dma_start(out=kbounce[:, :], in_=qkv[72:80, :])
                for u in range(4):
                    ksrc = bass.AP(tensor=kbounce.tensor,
                                   offset=kbounce.offset + u * 128,
                                   ap=[[NP, 8], [512, 8], [1, 128]])
                    nc.sync.dma_start(out=kr4[32 * u:32 * u + 8, 0:8, :],
                                      in_=ksrc)
                    bsrc = bass.AP(tensor=aux, offset=A_EB + u * 128,
                                   ap=[[512, 8], [1, 128]])
                    nc.sync.dma_start(out=kr4[32 * u + 8:32 * u + 9, 0:8, :],
                                      in_=bsrc)
                nc.sync.dma_start(out=kr4[0:8, 8, :], in_=kbounce[:, 4096:4224])
                nc.sync.dma_start(
                    out=kr4[8:9, 8, :],
                    in_=bass.AP(tensor=aux, offset=A_EB + 4096,
                                ap=[[NP, 1], [1, 128]]))

                # ---- vT transpose (+ones col), 4 per psum bank
                for j0 in range(0, 32, 4):
                    tp = mcp.tile([128, 4, 64], F32R, tag="mc",
                                  name=f"vtp{j0}")
                    for k in range(4):
                        jt = j0 + k
                        nc.tensor.transpose(
                            tp[:, k, :],
                            qkv[0:64, jt * 128:(jt + 1) * 128],
                            idt[0:64, 0:64])
                    nc.vector.tensor_copy(vT[:, j0:j0 + 4, 0:64], tp)
                tpl = mcp.tile([128, 64], F32R, tag="mc", name="vtpl")
                nc.tensor.transpose(tpl, qkv[0:64, 32 * 128:33 * 128],
                                    idt[0:64, 0:64])
                nc.vector.tensor_copy(vT[:, 32, 0:64], tpl)

                # ================= interleaved attention + CAM emission ========
                def pam_pair(jg0, chunk_cb=None):
                    """Emit energy/exp/pam for j-groups jg0, jg0+1 (or lone 8)."""
                    jgs = [jg0] if jg0 == 8 else [jg0, jg0 + 1]
                    for ici, (i0, iw) in enumerate(ICM):
                        pt = ptp.tile([65, iw], F32, tag="pt", name="pt")
                        nmm = sum(4 if j < 8 else 1 for j in jgs)
                        k = 0
                        for jg in jgs:
                            nu2 = 2 if jg < 8 else 1
                            for p in range(2 if jg < 8 else 1):
                                et_ps = ps.tile([128, 2, 512], F32, tag="ps",
                                                name="et_ps")
                                for u2 in range(nu2):
                                    u = 2 * p + u2
                                    nc.tensor.matmul(
                                        et_ps[:, u2, 0:iw],
                                        kr4[32 * u:32 * u + 32, jg, :],
                                        qr[32 * u:32 * u + 32, i0:i0 + iw],
                                        start=True, stop=True,
                                        tile_position=(32 * u, 0))
                                eT = etp.tile([128, 2, 512], F32R, tag="et",
                                              bufs=2, name="eT")
                                if nu2 == 2:
                                    nc.scalar.activation(eT[:, :, 0:iw],
                                                         et_ps[:, :, 0:iw],
                                                         AF.Exp, bias=0.0,
                                                         scale=1.0)
                                else:
                                    nc.scalar.activation(eT[:, 0, 0:iw],
                                                         et_ps[:, 0, 0:iw],
                                                         AF.Exp, bias=0.0,
                                                         scale=1.0)
                                for u2 in range(nu2):
                                    jt = 4 * jg + 2 * p + u2
                                    nc.tensor.matmul(pt, vT[:, jt, :],
                                                     eT[:, u2, 0:iw],
                                                     start=(k == 0),
                                                     stop=(k == nmm - 1))
                                    k += 1
                        if jg0 == 0:
                            nc.vector.tensor_copy(pacc[:, i0:i0 + iw], pt)
                        else:
                            nc.vector.tensor_tensor(pacc[:, i0:i0 + iw],
                                                    pacc[:, i0:i0 + iw], pt,
                                                    ALU.add)
                        if chunk_cb is not None:
                            chunk_cb(ici, i0, iw)

                pam_pair(0)
                # fT transposes (CAM input), masked
                for jt in range(NJT):
                    tp = mcp.tile([128, 64], F32R, tag="mc", name=f"ftp{jt}")
                    nc.tensor.transpose(tp, feat[0:64, jt * 128:(jt + 1) * 128],
                                        idt[0:64, 0:64])
                    nc.vector.tensor_scalar_mul(fT[:, jt, :], tp, nmt[:, jt:jt + 1])

                pam_pair(2)
                # CAM: ce (chunked), softmax, cattnT
                ce_sb = sm.tile([64, 64], F32, tag="ce_sb")
                for ci_, (j0, nj) in enumerate([(0, 9), (9, 8), (17, 8), (25, 8)]):
                    ce_ps = mcp.tile([64, 64], F32, tag="mc", name=f"ce{ci_}")
                    for k in range(nj):
                        jt = j0 + k
                        nc.tensor.matmul(ce_ps, fT[:, jt, :], fT[:, jt, :],
                                         start=(k == 0), stop=(k == nj - 1))
                    if ci_ == 0:
                        nc.vector.tensor_copy(ce_sb, ce_ps)
                    else:
                        nc.vector.tensor_tensor(ce_sb, ce_sb, ce_ps, ALU.add)
                rmin = sm.tile([64, 1], F32, tag="rmin")
                nc.vector.tensor_reduce(rmin, ce_sb, mybir.AxisListType.X, ALU.min)
                cu = sm.tile([64, 64], F32, tag="cu")
                nc.scalar.activation(cu, ce_sb, AF.Exp, bias=rmin, scale=-1.0)
                rs = sm.tile([64, 1], F32, tag="rs")
                nc.vector.tensor_reduce(rs, cu, mybir.AxisListType.X, ALU.add)
                nc.vector.reciprocal(rs, rs)
                cattn = sm.tile([64, 64], F32R, tag="cattn")
                nc.vector.tensor_scalar_mul(cattn, cu, rs)
                ctp = mcp.tile([64, 64], F32R, tag="mc", name="ctp")
                nc.tensor.transpose(ctp, cattn, idt[0:64, 0:64])
                cattnT = sm.tile([64, 64], F32R, tag="cattnT")
                nc.vector.tensor_copy(cattnT, ctp)

                pam_pair(4)
                # CAM apply + scbuf
                for (i0, iw) in IC:
                    cam_ps = mcp.tile([64, iw], F32, tag="mc", name="cam_ps")
                    nc.tensor.matmul(cam_ps, cattnT, feat[0:64, i0:i0 + iw],
                                     start=True, stop=True)
                    tmpc = etp.tile([64, iw], F32R, tag="camt", bufs=3,
                                    name="tmpc")
                    nc.vector.tensor_scalar_mul(tmpc, cam_ps, gcam)
                    r0, nr = i0 // W, iw // W
                    nc.vector.tensor_tensor(
                        scbuf[0:64, r0:r0 + nr, 1:65],
                        tmpc[:, :].rearrange("p (r c) -> p r c", c=W),
                        feat[0:64, i0:i0 + iw].rearrange("p (r c) -> p r c", c=W),
                        ALU.add)
                nc.vector.tensor_scalar_mul(scbuf[0:64, 0, 1:65],
                                            scbuf[0:64, 0, 1:65], hmt[:, 0:1])
                nc.vector.tensor_scalar_mul(scbuf[0:64, 33, 1:65],
                                            scbuf[0:64, 33, 1:65], hmt[:, 1:2])
                for (a, b) in [(0, 9), (9, 17), (17, 25), (25, 33)]:
                    nc.gpsimd.tensor_copy(scbuf[64:128, a:b, :],
                                          scbuf[0:64, a + 1:b + 1, :])

                def conv2(buf, y2sb, sttag):
                    st = sm.tile([64, 4, 6], F32, tag=sttag, name=sttag)
                    for T in range(4):
                        r0 = 1 + 8 * T
                        yps = mcp.tile([64, 512], F32, tag="mc", name="yps")
                        for dxi in range(3):
                            rhs1 = buf[:, r0 - 1:r0 + 7, dxi:dxi + 64]
                            nc.tensor.matmul(yps, w2at[:, dxi * 64:(dxi + 1) * 64],
                                             rhs1, start=(dxi == 0), stop=False)
                            rhs2 = buf[0:64, r0 + 1:r0 + 9, dxi:dxi + 64]
                            nc.tensor.matmul(yps, w2bt[:, dxi * 64:(dxi + 1) * 64],
                                             rhs2, start=False, stop=(dxi == 2))
                        nc.vector.bn_stats(st[:, T, :], yps)
                        nc.vector.tensor_copy(y2sb[:, T * 512:(T + 1) * 512], yps)
                    mv = sm.tile([64, 2], F32, tag=sttag + "mv", name=sttag + "mv")
                    nc.vector.bn_aggr(mv, st[:, :, :])
                    return mv

                pam_pair(6)
                # conv2 on CAM branch + its stats AR (hidden under attention)
                mvb = conv2(scbuf, y2b, "stb")
                glb = stat_ar(mvb, "arb")
                scb, shb = bn_coeffs(glb, "bnb")
                rb = big.tile([64, MY], F32R, tag="rb")
                nc.scalar.activation(rb, y2b, AF.Relu, bias=shb, scale=scb)

                # ---- pam normalize (r = gamma_pam / s), sa = pam_u*r + feat1
                def pam_div(src, i0, iw, sfx):
                    r32 = sm.tile([1, iw], F32, tag="r32", name="r32" + sfx)
                    nc.vector.reciprocal(r32, src[64:65, :])
                    rr = sm.tile([1, iw], F32R, tag="rr", name="rr" + sfx)
                    nc.vector.tensor_scalar_mul(rr, r32, cst[0:1, 0:1])
                    rbc = etp.tile([64, iw], F32R, tag="camt", bufs=3,
                                   name="rbc" + sfx)
                    nc.gpsimd.partition_broadcast(rbc, rr)
                    tmpa = etp.tile([64, iw], F32R, tag="camt", bufs=3,
                                    name="tmpa" + sfx)
                    nc.vector.tensor_tensor(tmpa, src[0:64, :], rbc, ALU.mult)
                    r0, nr = i0 // W, iw // W
                    nc.vector.tensor_tensor(
                        sabuf[0:64, r0:r0 + nr, 1:65],
                        tmpa[:, :].rearrange("p (r c) -> p r c", c=W),
                        feat[0:64, i0:i0 + iw].rearrange("p (r c) -> p r c", c=W),
                        ALU.add)

                pam_pair(8, chunk_cb=lambda ici, i0, iw: pam_div(
                    pacc[:, i0:i0 + iw], i0, iw, str(ici)))
                nc.vector.tensor_scalar_mul(sabuf[0:64, 0, 1:65],
                                            sabuf[0:64, 0, 1:65], hmt[:, 0:1])
                nc.vector.tensor_scalar_mul(sabuf[0:64, 33, 1:65],
                                            sabuf[0:64, 33, 1:65], hmt[:, 1:2])
                for (a, b) in [(0, 9), (9, 17), (17, 25), (25, 33)]:
                    nc.gpsimd.tensor_copy(sabuf[64:128, a:b, :],
                                          sabuf[0:64, a + 1:b + 1, :])

                mva = conv2(sabuf, y2a, "sta")
                gla = stat_ar(mva, "ara")
                sca, sha = bn_coeffs(gla, "bna")

                # ---- relu + sum -> fp16 feat_sum out (conv8 runs on host)
                for T in range(4):
                    sl = slice(T * 512, (T + 1) * 512)
                    ra = etp.tile([64, 512], F32R, tag="camt", bufs=3,
                                  name=f"ra{T}")
                    nc.scalar.activation(ra, y2a[:, sl], AF.Relu,
                                         bias=sha, scale=sca)
                    fo = etp.tile([64, 512], F16, tag="fo", bufs=3,
                                  name=f"fo{T}")
                    nc.vector.tensor_tensor(fo, ra, rb[:, sl], ALU.add)
                    nc.sync.dma_start(out=out[:, sl], in_=fo)

            for rep in range(nreps):
                _body(rep)
    nc.finalize()
    return nc


_NC_CACHE = {}


def kernel(**inputs):
    if "nc" not in _NC_CACHE:
        _NC_CACHE["nc"] = _build()
    nc = _NC_CACHE["nc"]
    x = np.asarray(inputs["x"], np.float32)
    w8 = np.asarray(inputs["w8"], np.float32)
    b8 = np.asarray(inputs["b8"], np.float32)
    in_maps = _prep_core_inputs(
        x, np.asarray(inputs["w1"]), np.asarray(inputs["bn_g"]),
        np.asarray(inputs["bn_b"]), np.asarray(inputs["wq"]),
        np.asarray(inputs["bq"]), np.asarray(inputs["wk"]),
        np.asarray(inputs["bk"]), np.asarray(inputs["wv"]),
        np.asarray(inputs["bv"]), np.asarray(inputs["gamma_pam"]),
        np.asarray(inputs["gamma_cam"]), np.asarray(inputs["w2"]),
        w8, b8)
    res = run_bass_kernel_spmd(nc, in_maps, list(range(NCORES)))
    fs = np.empty((B, CI, H * W), np.float32)
    for c in range(NCORES):
        b, h = divmod(c, 2)
        fs[b, :, MY * h:MY * h + MY] = res.results[c]["out"]
    W8 = w8[:, :, 0, 0]
    out = np.matmul(W8[None], fs) + b8[None, :, None]
    return out.reshape(B, CO, H, W).astype(np.float32)
